# revision 29
# baseline (speedup 1.0000x reference)
"""DeepseekV4 MLP (fp8-block-quantized gate_up/down, qdq activations, clamped
SwiGLU) on 8 Trainium2 NeuronCores.

Strategy: data-parallel over tokens (512 tokens/core), full weights streamed
per core.  BOTH matmul phases run in fp8 with MatmulPerfMode.DoubleRow
(2 fp8 weights/PE cell, 256-deep contraction per instruction, ~2x bf16).

  Phase A (gate_up): weights fold their power-of-two block scales as
  w' = e4m3(wq * s * 2^9) — an exact mantissa-preserving exponent shift into
  TRN float8e4 range (max 240) — and the 2^-9 descale rides the SwiGLU
  elementwise chain.  x8 = direct e4m3 cast of the reference's qdq'd
  activations (exact for all normal-range values: e4m3fn value x power-of-2
  scale keeps a 3-bit mantissa).  The DR pair-sum noise (~1.4e-4 relmax per
  matmul) perturbs gate_up, which flips some h-requantization roundings near
  grid midpoints; measured end-to-end headline error 1.64e-2 (vs 2.6e-3 for
  the bf16-phase-A variant), inside the 2e-2 gate, for ~2x phase-A
  throughput.  Measured on HW: a DoubleRow FD=512 matmul with real (dense
  random) operands costs ~210-240 ns — 1 moving col/cycle + 13%, i.e. the
  documented 157 TF/s fp8 peak, power-capped at ~2x bf16.  (With all-zero
  operands the same instruction streams 2 cols/cycle — a data/power
  artifact, not reachable with real data.)

  Phase B (down) identical to the bf16-A variant: w' = e4m3(wq * s * 2^9),
  h8 = direct e4m3 cast of h, DoubleRow k-pairs, 2^-9 descale on PSUM
  evacuation.

Dataflow (no on-device transposes): phase A computes gate/up TRANSPOSED:
psum[i-block, t] = sum_k w[k, i]^T @ x[k, t] (stationary = weight DR pair
[128,2,128], moving = x8 [128,2,512]).  SwiGLU runs elementwise on the [i, t]
psum pair and fp8 h lands directly in the [i_partition, t] layout phase B
needs as its stationary operand.  Phase B: psum[t, o] = h8[i, t]^T @
w_dn'[i, o] with DoubleRow k-pairs.
"""

import numpy as np
import ml_dtypes

import concourse.bass as bass
import concourse.mybir as mybir
import concourse.tile as tile
from concourse import bass_utils
from concourse.bass import ts

F32 = mybir.dt.float32
BF16 = mybir.dt.bfloat16
FP8 = mybir.dt.float8e4
AF = mybir.ActivationFunctionType
ALU = mybir.AluOpType
DR = mybir.MatmulPerfMode.DoubleRow

T, H, I = 4096, 4096, 11008
N_CORES = 8
TC = T // N_CORES            # 512 tokens per core
LIMIT = 7.0
BLOCK = 128
FP8_MAX = 448.0

WSHIFT = 512.0               # 2^9 folded into fp8 weights (both phases)
DESCALE = 1.0 / WSHIFT       # applied after each matmul phase

E4NP = ml_dtypes.float8_e4m3          # TRN float8e4: max 240
BF16NP = ml_dtypes.bfloat16


def build_nc(tc_tokens=TC, h=H, i_dim=I, waitfix=True, unroll=1,
             weights_internal=False, do_phases=(1, 1), a_reuse=1,
             a_jstride=0, a_skip_swiglu=0, a_swi=0, b_stream=1,
             ps_bufs=8, a_interleave=0, wap_bufs=4):
    """Per-core Bass program. DRAM shapes:
      x   [128, h/256, 2, tc] fp8          ([partition, k-pair, j, token])
      wa  [i/128, 128, 2, h/256, 2, 128] fp8 ([hb, p, gate|up, kp, j, i])
      wb  [h/512, 128, i/256, 2, 512] fp8  ([slab, p, kpair, j, o])
      out [tc, h] f32
    """
    ntt = tc_tokens // 128       # token tiles (phase B stationary)
    kpa = h // 256               # DR contraction pairs, gate_up (16)
    hbn = i_dim // 128           # intermediate blocks (86)
    kpb = hbn // 2               # DoubleRow k-pairs, down (43)
    slabs_b = h // 512           # down output slabs (8)

    nc = bass.Bass("TRN2", target_bir_lowering=False, debug=False, num_devices=1)
    wkind = "Internal" if weights_internal else "ExternalInput"
    wkind_b = "Internal" if weights_internal is True else "ExternalInput"
    x_d = nc.dram_tensor("x", [128, kpa, 2, tc_tokens], FP8, kind="ExternalInput")
    if a_jstride == 1:
        wa_d = nc.dram_tensor("wa", [hbn, 128, 2, 2, kpa, 128], FP8, kind=wkind)
    elif a_jstride == 2:
        wa_d = nc.dram_tensor("wa", [hbn, 128, 2, kpa // 4, 2, 4, 128], FP8, kind=wkind)
    else:
        wa_d = nc.dram_tensor("wa", [hbn, 128, 2, kpa, 2, 128], FP8, kind=wkind)
    wb_d = nc.dram_tensor("wb", [slabs_b, 128, kpb, 2, 512], FP8, kind=wkind_b)
    out_d = nc.dram_tensor("out", [tc_tokens, h], F32, kind="ExternalOutput")

    WB_CHUNK = 6                 # kpairs per wb DMA chunk

    with tile.TileContext(nc) as tc:
      for _rep in range(max(1, unroll)):
        with (
            tc.tile_pool(name="persist", bufs=1) as persist,
            tc.tile_pool(name="wb_pool", bufs=2) as wbp,
            tc.tile_pool(name="oev", bufs=2) as oev,
        ):
            h8 = persist.tile([128, hbn, tc_tokens], FP8)
            if not do_phases[0]:
                # timing-only B-isolation: initialize h8 so phase B can run
                nc.gpsimd.memset(h8[:], 0.25)

            wb_tiles = {}

            def issue_wb(s, engines=None):
                wbt = wbp.tile([128, kpb, 2, 512], FP8, tag="wb")
                if engines is None:
                    engines = ([nc.gpsimd, nc.scalar, nc.sync]
                               if b_stream else [nc.gpsimd, nc.scalar])
                for ci, c0 in enumerate(range(0, kpb, WB_CHUNK)):
                    cn = min(WB_CHUNK, kpb - c0)
                    eng = engines[ci % len(engines)]
                    eng.dma_start(
                        wbt[:, c0 : c0 + cn, :, :],
                        wb_d.ap()[s, :, c0 : c0 + cn, :, :],
                    )
                wb_tiles[s] = wbt

            # ---- Phase A: fp8 DoubleRow gate_up matmul (transposed) + swiglu -> h8 ----
            with (
                tc.tile_pool(name="pa", bufs=1) as pa,
                tc.tile_pool(name="wa_pool", bufs=wap_bufs) as wap,
                tc.tile_pool(name="psA", bufs=ps_bufs, space="PSUM") as psA,
                tc.tile_pool(name="swi", bufs=2) as swi,
            ):
                xT = pa.tile([128, kpa, 2, tc_tokens], FP8)
                xch = min(4, kpa)
                for xc in range(0, kpa, xch):
                    eng = nc.sync if (xc // xch) % 2 == 0 else nc.gpsimd
                    eng.dma_start(xT[:, xc : xc + xch, :, :],
                                  x_d.ap()[:, xc : xc + xch, :, :])
                # a_reuse>1: timing-only probe — same MM count, 1/a_reuse the
                # LDWEIGHTS (each stationary streamed a_reuse times)
                hbn_a = (hbn // a_reuse) if a_reuse > 1 else hbn
                for hb in range(hbn_a if do_phases[0] else 0):
                    wsh = ([128, 2, kpa, 128] if a_jstride == 1
                           else [128, kpa // 4, 2, 4, 128] if a_jstride == 2
                           else [128, kpa, 2, 128])
                    wat_g = wap.tile(wsh, FP8, tag="wa")
                    nc.sync.dma_start(wat_g[:, : wsh[1] // 2], wa_d.ap()[hb, :, 0, : wsh[1] // 2])
                    nc.gpsimd.dma_start(wat_g[:, wsh[1] // 2 :], wa_d.ap()[hb, :, 0, wsh[1] // 2 :])
                    wat_u = wap.tile(wsh, FP8, tag="wa")
                    nc.gpsimd.dma_start(wat_u[:, : wsh[1] // 2], wa_d.ap()[hb, :, 1, : wsh[1] // 2])
                    nc.sync.dma_start(wat_u[:, wsh[1] // 2 :], wa_d.ap()[hb, :, 1, wsh[1] // 2 :])
                    if b_stream:
                        # prefetch the first two down slabs on the quiet
                        # ACT/DVE rings so phase B starts with both resident
                        if hb == hbn - 12:
                            issue_wb(0, engines=[nc.scalar])
                        elif hb == hbn - 6:
                            issue_wb(1, engines=[nc.scalar])
                    elif hb == hbn - 8:
                        # prefetch first down slab on the quiet ACT ring so it
                        # cannot stall the phase-A weight streams
                        issue_wb(0, engines=[nc.scalar])
                    ps_g = psA.tile([128, tc_tokens], F32, tag="psA")
                    ps_u = psA.tile([128, tc_tokens], F32, tag="psA")
                    pairs = ((wat_g, ps_g), (wat_u, ps_u))
                    if a_interleave:
                        order = [(wat, ps, kp) for kp in range(kpa)
                                 for wat, ps in pairs]
                    else:
                        order = [(wat, ps, kp) for wat, ps in pairs
                                 for kp in range(kpa)]
                    for wat, ps, kp in order:
                        lhsT = (wat[:, :, kp, :] if a_jstride == 1
                                else wat[:, kp // 4, :, kp % 4, :] if a_jstride == 2
                                else wat[:, kp, :, :])
                        for r in range(a_reuse):
                            nc.tensor.matmul(
                                ps[:],
                                lhsT=lhsT,
                                rhs=xT[:, kp, :, :],
                                start=(kp == 0 and r == 0),
                                stop=(kp == kpa - 1 and r == a_reuse - 1),
                                perf_mode=(mybir.MatmulPerfMode.DoubleRowSwInterleave
                                           if a_swi else DR),
                            )
                    if a_skip_swiglu:
                        junk = swi.tile([128, tc_tokens], F32, tag="junk")
                        nc.vector.tensor_tensor(
                            out=junk[:], in0=ps_g[:], in1=ps_u[:], op=ALU.mult)
                        continue
                    # SwiGLU on the [i(128), t(tc)] psum pair; psum carries
                    # gate_up * 2^9 (weight fold), descale rides the chain:
                    #   gc = min(ps_g*D^2, 7*D)        = D*min(g,7)
                    #   sg = sigmoid(gc * 1/D)         = sigmoid(min(g,7))
                    #   uc = clip(ps_u, +-7/D)         = (1/D)*clip(u,+-7)
                    #   gc = gc*sg;  h8 = gc*uc        = silu(min(g,7))*clip(u,+-7)
                    gc = swi.tile([128, tc_tokens], F32, tag="gc")
                    nc.vector.tensor_scalar(
                        out=gc[:], in0=ps_g[:],
                        scalar1=DESCALE * DESCALE, scalar2=LIMIT * DESCALE,
                        op0=ALU.mult, op1=ALU.min,
                    )
                    sg = swi.tile([128, tc_tokens], F32, tag="sg")
                    nc.scalar.activation(sg[:], gc[:], AF.Sigmoid, scale=WSHIFT)
                    uc = swi.tile([128, tc_tokens], F32, tag="uc")
                    nc.vector.tensor_scalar(
                        out=uc[:], in0=ps_u[:],
                        scalar1=LIMIT * WSHIFT, scalar2=-LIMIT * WSHIFT,
                        op0=ALU.min, op1=ALU.max,
                    )
                    nc.vector.tensor_mul(gc[:], gc[:], sg[:])
                    nc.vector.tensor_tensor(
                        out=h8[:, hb, :], in0=gc[:], in1=uc[:], op=ALU.mult,
                    )

            # ---- Phase B: fp8 DoubleRow down matmul ----
            with tc.tile_pool(name="psB", bufs=ps_bufs, space="PSUM") as psB:
                for s in range(slabs_b if do_phases[1] else 0):
                    if s not in wb_tiles:
                        issue_wb(s)
                    if s + 1 < slabs_b and (s + 1) not in wb_tiles:
                        issue_wb(s + 1)
                    wbt = wb_tiles.pop(s)
                    for tt in range(ntt):
                        ps = psB.tile([128, 512], F32, tag="psB")
                        for kp in range(kpb):
                            nc.tensor.matmul(
                                ps[:],
                                lhsT=h8[:, 2 * kp : 2 * kp + 2, ts(tt, 128)],
                                rhs=wbt[:, kp, :, :],
                                start=(kp == 0), stop=(kp == kpb - 1),
                                perf_mode=DR,
                            )
                        ot = oev.tile([128, 512], F32, tag="ot")
                        nc.scalar.activation(
                            ot[:], ps[:], AF.Copy, bias=0.0, scale=DESCALE,
                        )
                        nc.sync.dma_start(out_d.ap()[ts(tt, 128), ts(s, 512)], ot[:])

    if waitfix:
        from waitfix import split_multi_waits
        split_multi_waits(nc)
    return nc


# waitfix inlined so kernel.py stays self-contained
import sys as _sys
import types as _types

if "waitfix" not in _sys.modules:
    _wf = _types.ModuleType("waitfix")

    def _split_multi_waits(nc, limit: int = 1) -> int:
        n_split = 0
        f = nc.m.functions[0]
        for blk in f.blocks:
            insts = blk.instructions  # live list
            i = 0
            while i < len(insts):
                ins = insts[i]
                si = ins.sync_info
                if si is not None and len(si.on_wait) > limit:
                    waits = list(si.on_wait)
                    keep = waits[-limit:]
                    extra = waits[:-limit]
                    new_nops = []
                    for w in extra:
                        nop = mybir.InstNoOp(name=f"WSPLIT-{nc.next_id()}", ins=[], outs=[])
                        nop.engine = ins.engine
                        nop.sync_info = mybir.SyncInfo(on_wait=[w], on_update=[])
                        new_nops.append(nop)
                    ins.sync_info = mybir.SyncInfo(on_wait=keep, on_update=list(si.on_update))
                    for j, nop in enumerate(new_nops):
                        insts.insert(i + j, nop)
                    i += len(new_nops)
                    n_split += 1
                i += 1
        return n_split

    _wf.split_multi_waits = _split_multi_waits
    _sys.modules["waitfix"] = _wf


def _fold_w_fp8(w, s, block=BLOCK):
    """w [O, K] f32 (e4m3-representable values), s [O/128, K/128] pow-2 scales
    -> e4m3(w * s * 2^9) as float8_e4m3 [O, K].  Mantissa-exact shift."""
    ob, ib = s.shape
    w4 = w.reshape(ob, block, ib, block) * (s[:, None, :, None] * WSHIFT)
    w4 = w4.reshape(ob * block, ib * block)
    return np.clip(w4, -240.0, 240.0).astype(E4NP)


def _act_qdq_bf16(x):
    """Bit-exact replica of reference._act_qdq (jax cpu ops), cast to bf16
    (exact: e4m3 value x power-of-2 scale fits bf16's 8-bit mantissa)."""
    import jax
    import jax.numpy as jnp
    cpu = jax.devices("cpu")[0]
    with jax.default_device(cpu):
        xb = jnp.asarray(x, jnp.float32).reshape(-1, x.shape[-1] // BLOCK, BLOCK)
        amax = jnp.maximum(jnp.max(jnp.abs(xb), axis=-1), 1e-4)
        scale = jnp.exp2(jnp.ceil(jnp.log2(amax / FP8_MAX)))
        q = jnp.clip(xb / scale[..., None], -FP8_MAX, FP8_MAX)
        q = q.astype(jnp.float8_e4m3fn).astype(jnp.float32)
        out = np.asarray(q * scale[..., None]).reshape(x.shape)
    return out.astype(BF16NP)


def prep_weights(w_gate_up, s_gate_up, w_down, s_down, h=H, i_dim=I):
    """Host-side layouts (see build_nc docstring)."""
    hbn = i_dim // 128
    kpa = h // 256
    kpb = hbn // 2
    slabs_b = h // 512

    wA8 = _fold_w_fp8(w_gate_up, s_gate_up)             # [2I, H] fp8
    # [gu, hb, i, kp, j, p] -> [hb, p, gu, kp, j, i]
    wa = wA8.reshape(2, hbn, 128, kpa, 2, 128).transpose(1, 5, 0, 3, 4, 2)
    wa = np.ascontiguousarray(wa)

    wB8 = _fold_w_fp8(w_down, s_down)                   # [H, I] fp8
    # [s, o, kp, j, p] -> [s, p, kp, j, o]
    wb = wB8.reshape(slabs_b, 512, kpb, 2, 128).transpose(0, 4, 2, 3, 1)
    wb = np.ascontiguousarray(wb)
    return wa, wb


def prep_x(xq, h=H):
    """qdq'd x [T', h] bf16 -> e4m3 (exact for normal-range values) in the
    DR-paired [128, h/256, 2, T'] transposed layout."""
    tcn = xq.shape[0]
    x8 = xq.astype(np.float32).astype(E4NP)
    return np.ascontiguousarray(
        x8.reshape(tcn, h // 256, 2, 128).transpose(3, 1, 2, 0))


def prep_inputs(inputs):
    """Full input dict -> per-core in_maps."""
    x = np.asarray(inputs["x"], np.float32)
    xq = _act_qdq_bf16(x)
    wa, wb = prep_weights(
        np.asarray(inputs["w_gate_up"], np.float32),
        np.asarray(inputs["s_gate_up"], np.float32),
        np.asarray(inputs["w_down"], np.float32),
        np.asarray(inputs["s_down"], np.float32),
    )
    return [
        {"x": prep_x(xq[c * TC : (c + 1) * TC]), "wa": wa, "wb": wb}
        for c in range(N_CORES)
    ]


_CACHE = {}


def kernel(x, w_gate_up, s_gate_up, w_down, s_down):
    in_maps = prep_inputs(dict(x=x, w_gate_up=w_gate_up, s_gate_up=s_gate_up,
                               w_down=w_down, s_down=s_down))
    if "nc" not in _CACHE:
        _CACHE["nc"] = build_nc()
    nc = _CACHE["nc"]
    res = bass_utils.run_bass_kernel_spmd(nc, in_maps, core_ids=list(range(N_CORES)))
    return np.concatenate([res.results[c]["out"] for c in range(N_CORES)], axis=0)


# revision 30
# speedup vs baseline: 1.0241x; 1.0241x over previous
"""DeepseekV4 MLP (fp8-block-quantized gate_up/down, qdq activations, clamped
SwiGLU) on 8 Trainium2 NeuronCores.

Strategy: data-parallel over tokens (512 tokens/core), full weights streamed
per core.  BOTH matmul phases run in fp8 with MatmulPerfMode.DoubleRow
(2 fp8 weights/PE cell, 256-deep contraction per instruction, ~2x bf16).

  Phase A (gate_up): weights fold their power-of-two block scales as
  w' = e4m3(wq * s * 2^9) — an exact mantissa-preserving exponent shift into
  TRN float8e4 range (max 240) — and the 2^-9 descale rides the SwiGLU
  elementwise chain.  x8 = direct e4m3 cast of the reference's qdq'd
  activations (exact for all normal-range values: e4m3fn value x power-of-2
  scale keeps a 3-bit mantissa).  The DR pair-sum noise (~1.4e-4 relmax per
  matmul) perturbs gate_up, which flips some h-requantization roundings near
  grid midpoints; measured end-to-end headline error 1.64e-2 (vs 2.6e-3 for
  the bf16-phase-A variant), inside the 2e-2 gate, for ~2x phase-A
  throughput.  Measured on HW: a DoubleRow FD=512 matmul with real (dense
  random) operands costs ~210-240 ns — 1 moving col/cycle + 13%, i.e. the
  documented 157 TF/s fp8 peak, power-capped at ~2x bf16.  (With all-zero
  operands the same instruction streams 2 cols/cycle — a data/power
  artifact, not reachable with real data.)

  Phase B (down) identical to the bf16-A variant: w' = e4m3(wq * s * 2^9),
  h8 = direct e4m3 cast of h, DoubleRow k-pairs, 2^-9 descale on PSUM
  evacuation.

Dataflow (no on-device transposes): phase A computes gate/up TRANSPOSED:
psum[i-block, t] = sum_k w[k, i]^T @ x[k, t] (stationary = weight DR pair
[128,2,128], moving = x8 [128,2,512]).  SwiGLU runs elementwise on the [i, t]
psum pair and fp8 h lands directly in the [i_partition, t] layout phase B
needs as its stationary operand.  Phase B: psum[t, o] = h8[i, t]^T @
w_dn'[i, o] with DoubleRow k-pairs.
"""

import numpy as np
import ml_dtypes

import concourse.bass as bass
import concourse.mybir as mybir
import concourse.tile as tile
from concourse import bass_utils
from concourse.bass import ts

F32 = mybir.dt.float32
BF16 = mybir.dt.bfloat16
FP8 = mybir.dt.float8e4
AF = mybir.ActivationFunctionType
ALU = mybir.AluOpType
DR = mybir.MatmulPerfMode.DoubleRow

T, H, I = 4096, 4096, 11008
N_CORES = 8
TC = T // N_CORES            # 512 tokens per core
LIMIT = 7.0
BLOCK = 128
FP8_MAX = 448.0

WSHIFT = 512.0               # 2^9 folded into fp8 weights (both phases)
DESCALE = 1.0 / WSHIFT       # applied after each matmul phase

E4NP = ml_dtypes.float8_e4m3          # TRN float8e4: max 240
BF16NP = ml_dtypes.bfloat16


def build_nc(tc_tokens=TC, h=H, i_dim=I, waitfix=True, unroll=1,
             weights_internal=False, do_phases=(1, 1), a_reuse=1,
             a_jstride=0, a_skip_swiglu=0, a_swi=0, b_stream=1,
             ps_bufs=8, a_interleave=0, wap_bufs=4,
             wb_chunk=6, swi_bufs=2, wa3ring=0):
    """Per-core Bass program. DRAM shapes:
      x   [128, h/256, 2, tc] fp8          ([partition, k-pair, j, token])
      wa  [i/128, 128, 2, h/256, 2, 128] fp8 ([hb, p, gate|up, kp, j, i])
      wb  [h/512, 128, i/256, 2, 512] fp8  ([slab, p, kpair, j, o])
      out [tc, h] f32
    """
    ntt = tc_tokens // 128       # token tiles (phase B stationary)
    kpa = h // 256               # DR contraction pairs, gate_up (16)
    hbn = i_dim // 128           # intermediate blocks (86)
    kpb = hbn // 2               # DoubleRow k-pairs, down (43)
    slabs_b = h // 512           # down output slabs (8)

    nc = bass.Bass("TRN2", target_bir_lowering=False, debug=False, num_devices=1)
    wkind = "Internal" if weights_internal else "ExternalInput"
    wkind_b = "Internal" if weights_internal is True else "ExternalInput"
    x_d = nc.dram_tensor("x", [128, kpa, 2, tc_tokens], FP8, kind="ExternalInput")
    if a_jstride == 1:
        wa_d = nc.dram_tensor("wa", [hbn, 128, 2, 2, kpa, 128], FP8, kind=wkind)
    elif a_jstride == 2:
        wa_d = nc.dram_tensor("wa", [hbn, 128, 2, kpa // 4, 2, 4, 128], FP8, kind=wkind)
    else:
        wa_d = nc.dram_tensor("wa", [hbn, 128, 2, kpa, 2, 128], FP8, kind=wkind)
    wb_d = nc.dram_tensor("wb", [slabs_b, 128, kpb, 2, 512], FP8, kind=wkind_b)
    out_d = nc.dram_tensor("out", [tc_tokens, h], F32, kind="ExternalOutput")

    WB_CHUNK = wb_chunk          # kpairs per wb DMA chunk

    with tile.TileContext(nc) as tc:
      for _rep in range(max(1, unroll)):
        with (
            tc.tile_pool(name="persist", bufs=1) as persist,
            tc.tile_pool(name="wb_pool", bufs=2) as wbp,
            tc.tile_pool(name="oev", bufs=2) as oev,
        ):
            h8 = persist.tile([128, hbn, tc_tokens], FP8)
            if not do_phases[0]:
                # timing-only B-isolation: initialize h8 so phase B can run
                nc.gpsimd.memset(h8[:], 0.25)

            wb_tiles = {}

            def issue_wb(s, engines=None):
                wbt = wbp.tile([128, kpb, 2, 512], FP8, tag="wb")
                if engines is None:
                    engines = ([nc.gpsimd, nc.scalar, nc.sync]
                               if b_stream else [nc.gpsimd, nc.scalar])
                for ci, c0 in enumerate(range(0, kpb, WB_CHUNK)):
                    cn = min(WB_CHUNK, kpb - c0)
                    eng = engines[ci % len(engines)]
                    eng.dma_start(
                        wbt[:, c0 : c0 + cn, :, :],
                        wb_d.ap()[s, :, c0 : c0 + cn, :, :],
                    )
                wb_tiles[s] = wbt

            # ---- Phase A: fp8 DoubleRow gate_up matmul (transposed) + swiglu -> h8 ----
            with (
                tc.tile_pool(name="pa", bufs=1) as pa,
                tc.tile_pool(name="wa_pool", bufs=wap_bufs) as wap,
                tc.tile_pool(name="psA", bufs=ps_bufs, space="PSUM") as psA,
                tc.tile_pool(name="swi", bufs=swi_bufs) as swi,
            ):
                xT = pa.tile([128, kpa, 2, tc_tokens], FP8)
                xch = min(4, kpa)
                for xc in range(0, kpa, xch):
                    eng = nc.sync if (xc // xch) % 2 == 0 else nc.gpsimd
                    eng.dma_start(xT[:, xc : xc + xch, :, :],
                                  x_d.ap()[:, xc : xc + xch, :, :])
                # a_reuse>1: timing-only probe — same MM count, 1/a_reuse the
                # LDWEIGHTS (each stationary streamed a_reuse times)
                hbn_a = (hbn // a_reuse) if a_reuse > 1 else hbn
                for hb in range(hbn_a if do_phases[0] else 0):
                    wsh = ([128, 2, kpa, 128] if a_jstride == 1
                           else [128, kpa // 4, 2, 4, 128] if a_jstride == 2
                           else [128, kpa, 2, 128])
                    wat_g = wap.tile(wsh, FP8, tag="wa")
                    wat_u = wap.tile(wsh, FP8, tag="wa")
                    if wa3ring and not (b_stream and hbn - 12 <= hb <= hbn - 5):
                        # 3-way split outside the wb-prefetch window
                        c3 = wsh[1] // 4
                        for wat, g in ((wat_g, 0), (wat_u, 1)):
                            nc.sync.dma_start(wat[:, : c3], wa_d.ap()[hb, :, g, : c3])
                            nc.gpsimd.dma_start(wat[:, c3 : 2 * c3], wa_d.ap()[hb, :, g, c3 : 2 * c3])
                            nc.scalar.dma_start(wat[:, 2 * c3 :], wa_d.ap()[hb, :, g, 2 * c3 :])
                    else:
                        nc.sync.dma_start(wat_g[:, : wsh[1] // 2], wa_d.ap()[hb, :, 0, : wsh[1] // 2])
                        nc.gpsimd.dma_start(wat_g[:, wsh[1] // 2 :], wa_d.ap()[hb, :, 0, wsh[1] // 2 :])
                        nc.gpsimd.dma_start(wat_u[:, : wsh[1] // 2], wa_d.ap()[hb, :, 1, : wsh[1] // 2])
                        nc.sync.dma_start(wat_u[:, wsh[1] // 2 :], wa_d.ap()[hb, :, 1, wsh[1] // 2 :])
                    if b_stream:
                        # prefetch the first two down slabs on the quiet
                        # ACT/DVE rings so phase B starts with both resident
                        if hb == hbn - 12:
                            issue_wb(0, engines=[nc.scalar])
                        elif hb == hbn - 6:
                            issue_wb(1, engines=[nc.scalar])
                    elif hb == hbn - 8:
                        # prefetch first down slab on the quiet ACT ring so it
                        # cannot stall the phase-A weight streams
                        issue_wb(0, engines=[nc.scalar])
                    ps_g = psA.tile([128, tc_tokens], F32, tag="psA")
                    ps_u = psA.tile([128, tc_tokens], F32, tag="psA")
                    pairs = ((wat_g, ps_g), (wat_u, ps_u))
                    if a_interleave:
                        order = [(wat, ps, kp) for kp in range(kpa)
                                 for wat, ps in pairs]
                    else:
                        order = [(wat, ps, kp) for wat, ps in pairs
                                 for kp in range(kpa)]
                    for wat, ps, kp in order:
                        lhsT = (wat[:, :, kp, :] if a_jstride == 1
                                else wat[:, kp // 4, :, kp % 4, :] if a_jstride == 2
                                else wat[:, kp, :, :])
                        for r in range(a_reuse):
                            nc.tensor.matmul(
                                ps[:],
                                lhsT=lhsT,
                                rhs=xT[:, kp, :, :],
                                start=(kp == 0 and r == 0),
                                stop=(kp == kpa - 1 and r == a_reuse - 1),
                                perf_mode=(mybir.MatmulPerfMode.DoubleRowSwInterleave
                                           if a_swi else DR),
                            )
                    if a_skip_swiglu:
                        junk = swi.tile([128, tc_tokens], F32, tag="junk")
                        nc.vector.tensor_tensor(
                            out=junk[:], in0=ps_g[:], in1=ps_u[:], op=ALU.mult)
                        continue
                    # SwiGLU on the [i(128), t(tc)] psum pair; psum carries
                    # gate_up * 2^9 (weight fold), descale rides the chain:
                    #   gc = min(ps_g*D^2, 7*D)        = D*min(g,7)
                    #   sg = sigmoid(gc * 1/D)         = sigmoid(min(g,7))
                    #   uc = clip(ps_u, +-7/D)         = (1/D)*clip(u,+-7)
                    #   gc = gc*sg;  h8 = gc*uc        = silu(min(g,7))*clip(u,+-7)
                    gc = swi.tile([128, tc_tokens], F32, tag="gc")
                    nc.vector.tensor_scalar(
                        out=gc[:], in0=ps_g[:],
                        scalar1=DESCALE * DESCALE, scalar2=LIMIT * DESCALE,
                        op0=ALU.mult, op1=ALU.min,
                    )
                    sg = swi.tile([128, tc_tokens], F32, tag="sg")
                    nc.scalar.activation(sg[:], gc[:], AF.Sigmoid, scale=WSHIFT)
                    uc = swi.tile([128, tc_tokens], F32, tag="uc")
                    nc.vector.tensor_scalar(
                        out=uc[:], in0=ps_u[:],
                        scalar1=LIMIT * WSHIFT, scalar2=-LIMIT * WSHIFT,
                        op0=ALU.min, op1=ALU.max,
                    )
                    nc.vector.tensor_mul(gc[:], gc[:], sg[:])
                    nc.vector.tensor_tensor(
                        out=h8[:, hb, :], in0=gc[:], in1=uc[:], op=ALU.mult,
                    )

            # ---- Phase B: fp8 DoubleRow down matmul ----
            with tc.tile_pool(name="psB", bufs=ps_bufs, space="PSUM") as psB:
                for s in range(slabs_b if do_phases[1] else 0):
                    if s not in wb_tiles:
                        issue_wb(s)
                    if s + 1 < slabs_b and (s + 1) not in wb_tiles:
                        issue_wb(s + 1)
                    wbt = wb_tiles.pop(s)
                    for tt in range(ntt):
                        ps = psB.tile([128, 512], F32, tag="psB")
                        for kp in range(kpb):
                            nc.tensor.matmul(
                                ps[:],
                                lhsT=h8[:, 2 * kp : 2 * kp + 2, ts(tt, 128)],
                                rhs=wbt[:, kp, :, :],
                                start=(kp == 0), stop=(kp == kpb - 1),
                                perf_mode=DR,
                            )
                        ot = oev.tile([128, 512], F32, tag="ot")
                        nc.scalar.activation(
                            ot[:], ps[:], AF.Copy, bias=0.0, scale=DESCALE,
                        )
                        nc.sync.dma_start(out_d.ap()[ts(tt, 128), ts(s, 512)], ot[:])

    if waitfix:
        from waitfix import split_multi_waits
        split_multi_waits(nc)
    return nc


# waitfix inlined so kernel.py stays self-contained
import sys as _sys
import types as _types

if "waitfix" not in _sys.modules:
    _wf = _types.ModuleType("waitfix")

    def _split_multi_waits(nc, limit: int = 1) -> int:
        n_split = 0
        f = nc.m.functions[0]
        for blk in f.blocks:
            insts = blk.instructions  # live list
            i = 0
            while i < len(insts):
                ins = insts[i]
                si = ins.sync_info
                if si is not None and len(si.on_wait) > limit:
                    waits = list(si.on_wait)
                    keep = waits[-limit:]
                    extra = waits[:-limit]
                    new_nops = []
                    for w in extra:
                        nop = mybir.InstNoOp(name=f"WSPLIT-{nc.next_id()}", ins=[], outs=[])
                        nop.engine = ins.engine
                        nop.sync_info = mybir.SyncInfo(on_wait=[w], on_update=[])
                        new_nops.append(nop)
                    ins.sync_info = mybir.SyncInfo(on_wait=keep, on_update=list(si.on_update))
                    for j, nop in enumerate(new_nops):
                        insts.insert(i + j, nop)
                    i += len(new_nops)
                    n_split += 1
                i += 1
        return n_split

    _wf.split_multi_waits = _split_multi_waits
    _sys.modules["waitfix"] = _wf


def _fold_w_fp8(w, s, block=BLOCK):
    """w [O, K] f32 (e4m3-representable values), s [O/128, K/128] pow-2 scales
    -> e4m3(w * s * 2^9) as float8_e4m3 [O, K].  Mantissa-exact shift."""
    ob, ib = s.shape
    w4 = w.reshape(ob, block, ib, block) * (s[:, None, :, None] * WSHIFT)
    w4 = w4.reshape(ob * block, ib * block)
    return np.clip(w4, -240.0, 240.0).astype(E4NP)


def _act_qdq_bf16(x):
    """Bit-exact replica of reference._act_qdq (jax cpu ops), cast to bf16
    (exact: e4m3 value x power-of-2 scale fits bf16's 8-bit mantissa)."""
    import jax
    import jax.numpy as jnp
    cpu = jax.devices("cpu")[0]
    with jax.default_device(cpu):
        xb = jnp.asarray(x, jnp.float32).reshape(-1, x.shape[-1] // BLOCK, BLOCK)
        amax = jnp.maximum(jnp.max(jnp.abs(xb), axis=-1), 1e-4)
        scale = jnp.exp2(jnp.ceil(jnp.log2(amax / FP8_MAX)))
        q = jnp.clip(xb / scale[..., None], -FP8_MAX, FP8_MAX)
        q = q.astype(jnp.float8_e4m3fn).astype(jnp.float32)
        out = np.asarray(q * scale[..., None]).reshape(x.shape)
    return out.astype(BF16NP)


def prep_weights(w_gate_up, s_gate_up, w_down, s_down, h=H, i_dim=I):
    """Host-side layouts (see build_nc docstring)."""
    hbn = i_dim // 128
    kpa = h // 256
    kpb = hbn // 2
    slabs_b = h // 512

    wA8 = _fold_w_fp8(w_gate_up, s_gate_up)             # [2I, H] fp8
    # [gu, hb, i, kp, j, p] -> [hb, p, gu, kp, j, i]
    wa = wA8.reshape(2, hbn, 128, kpa, 2, 128).transpose(1, 5, 0, 3, 4, 2)
    wa = np.ascontiguousarray(wa)

    wB8 = _fold_w_fp8(w_down, s_down)                   # [H, I] fp8
    # [s, o, kp, j, p] -> [s, p, kp, j, o]
    wb = wB8.reshape(slabs_b, 512, kpb, 2, 128).transpose(0, 4, 2, 3, 1)
    wb = np.ascontiguousarray(wb)
    return wa, wb


def prep_x(xq, h=H):
    """qdq'd x [T', h] bf16 -> e4m3 (exact for normal-range values) in the
    DR-paired [128, h/256, 2, T'] transposed layout."""
    tcn = xq.shape[0]
    x8 = xq.astype(np.float32).astype(E4NP)
    return np.ascontiguousarray(
        x8.reshape(tcn, h // 256, 2, 128).transpose(3, 1, 2, 0))


def prep_inputs(inputs):
    """Full input dict -> per-core in_maps."""
    x = np.asarray(inputs["x"], np.float32)
    xq = _act_qdq_bf16(x)
    wa, wb = prep_weights(
        np.asarray(inputs["w_gate_up"], np.float32),
        np.asarray(inputs["s_gate_up"], np.float32),
        np.asarray(inputs["w_down"], np.float32),
        np.asarray(inputs["s_down"], np.float32),
    )
    return [
        {"x": prep_x(xq[c * TC : (c + 1) * TC]), "wa": wa, "wb": wb}
        for c in range(N_CORES)
    ]


_CACHE = {}


def kernel(x, w_gate_up, s_gate_up, w_down, s_down):
    in_maps = prep_inputs(dict(x=x, w_gate_up=w_gate_up, s_gate_up=s_gate_up,
                               w_down=w_down, s_down=s_down))
    if "nc" not in _CACHE:
        _CACHE["nc"] = build_nc()
    nc = _CACHE["nc"]
    res = bass_utils.run_bass_kernel_spmd(nc, in_maps, core_ids=list(range(N_CORES)))
    return np.concatenate([res.results[c]["out"] for c in range(N_CORES)], axis=0)


# revision 31
# speedup vs baseline: 1.0483x; 1.0237x over previous
"""DeepseekV4 MLP (fp8-block-quantized gate_up/down, qdq activations, clamped
SwiGLU) on 8 Trainium2 NeuronCores.

Strategy: data-parallel over tokens (512 tokens/core), full weights streamed
per core.  BOTH matmul phases run in fp8 with MatmulPerfMode.DoubleRow
(2 fp8 weights/PE cell, 256-deep contraction per instruction, ~2x bf16).

  Phase A (gate_up): weights fold their power-of-two block scales as
  w' = e4m3(wq * s * 2^9) — an exact mantissa-preserving exponent shift into
  TRN float8e4 range (max 240) — and the 2^-9 descale rides the SwiGLU
  elementwise chain.  x8 = direct e4m3 cast of the reference's qdq'd
  activations (exact for all normal-range values: e4m3fn value x power-of-2
  scale keeps a 3-bit mantissa).  The DR pair-sum noise (~1.4e-4 relmax per
  matmul) perturbs gate_up, which flips some h-requantization roundings near
  grid midpoints; measured end-to-end headline error 1.64e-2 (vs 2.6e-3 for
  the bf16-phase-A variant), inside the 2e-2 gate, for ~2x phase-A
  throughput.  Measured on HW: a DoubleRow FD=512 matmul with real (dense
  random) operands costs ~210-240 ns — 1 moving col/cycle + 13%, i.e. the
  documented 157 TF/s fp8 peak, power-capped at ~2x bf16.  (With all-zero
  operands the same instruction streams 2 cols/cycle — a data/power
  artifact, not reachable with real data.)

  Phase B (down) identical to the bf16-A variant: w' = e4m3(wq * s * 2^9),
  h8 = direct e4m3 cast of h, DoubleRow k-pairs, 2^-9 descale on PSUM
  evacuation.

Dataflow (no on-device transposes): phase A computes gate/up TRANSPOSED:
psum[i-block, t] = sum_k w[k, i]^T @ x[k, t] (stationary = weight DR pair
[128,2,128], moving = x8 [128,2,512]).  SwiGLU runs elementwise on the [i, t]
psum pair and fp8 h lands directly in the [i_partition, t] layout phase B
needs as its stationary operand.  Phase B: psum[t, o] = h8[i, t]^T @
w_dn'[i, o] with DoubleRow k-pairs.
"""

import numpy as np
import ml_dtypes

import concourse.bass as bass
import concourse.mybir as mybir
import concourse.tile as tile
from concourse import bass_utils
from concourse.bass import ts

F32 = mybir.dt.float32
BF16 = mybir.dt.bfloat16
FP8 = mybir.dt.float8e4
AF = mybir.ActivationFunctionType
ALU = mybir.AluOpType
DR = mybir.MatmulPerfMode.DoubleRow

T, H, I = 4096, 4096, 11008
N_CORES = 8
TC = T // N_CORES            # 512 tokens per core
LIMIT = 7.0
BLOCK = 128
FP8_MAX = 448.0

WSHIFT = 512.0               # 2^9 folded into fp8 weights (both phases)
DESCALE = 1.0 / WSHIFT       # applied after each matmul phase

E4NP = ml_dtypes.float8_e4m3          # TRN float8e4: max 240
BF16NP = ml_dtypes.bfloat16


def build_nc(tc_tokens=TC, h=H, i_dim=I, waitfix=True, unroll=1,
             weights_internal=False, do_phases=(1, 1), a_reuse=1,
             a_jstride=0, a_skip_swiglu=0, a_swi=0, b_stream=1,
             ps_bufs=8, a_interleave=0, wap_bufs=4,
             wb_chunk=4, swi_bufs=3, wa3ring=1):
    """Per-core Bass program. DRAM shapes:
      x   [128, h/256, 2, tc] fp8          ([partition, k-pair, j, token])
      wa  [i/128, 128, 2, h/256, 2, 128] fp8 ([hb, p, gate|up, kp, j, i])
      wb  [h/512, 128, i/256, 2, 512] fp8  ([slab, p, kpair, j, o])
      out [tc, h] f32
    """
    ntt = tc_tokens // 128       # token tiles (phase B stationary)
    kpa = h // 256               # DR contraction pairs, gate_up (16)
    hbn = i_dim // 128           # intermediate blocks (86)
    kpb = hbn // 2               # DoubleRow k-pairs, down (43)
    slabs_b = h // 512           # down output slabs (8)

    nc = bass.Bass("TRN2", target_bir_lowering=False, debug=False, num_devices=1)
    wkind = "Internal" if weights_internal else "ExternalInput"
    wkind_b = "Internal" if weights_internal is True else "ExternalInput"
    x_d = nc.dram_tensor("x", [128, kpa, 2, tc_tokens], FP8, kind="ExternalInput")
    if a_jstride == 1:
        wa_d = nc.dram_tensor("wa", [hbn, 128, 2, 2, kpa, 128], FP8, kind=wkind)
    elif a_jstride == 2:
        wa_d = nc.dram_tensor("wa", [hbn, 128, 2, kpa // 4, 2, 4, 128], FP8, kind=wkind)
    else:
        wa_d = nc.dram_tensor("wa", [hbn, 128, 2, kpa, 2, 128], FP8, kind=wkind)
    wb_d = nc.dram_tensor("wb", [slabs_b, 128, kpb, 2, 512], FP8, kind=wkind_b)
    out_d = nc.dram_tensor("out", [tc_tokens, h], F32, kind="ExternalOutput")

    WB_CHUNK = wb_chunk          # kpairs per wb DMA chunk

    with tile.TileContext(nc) as tc:
      for _rep in range(max(1, unroll)):
        with (
            tc.tile_pool(name="persist", bufs=1) as persist,
            tc.tile_pool(name="wb_pool", bufs=2) as wbp,
            tc.tile_pool(name="oev", bufs=2) as oev,
        ):
            h8 = persist.tile([128, hbn, tc_tokens], FP8)
            if not do_phases[0]:
                # timing-only B-isolation: initialize h8 so phase B can run
                nc.gpsimd.memset(h8[:], 0.25)

            wb_tiles = {}

            def issue_wb(s, engines=None):
                wbt = wbp.tile([128, kpb, 2, 512], FP8, tag="wb")
                if engines is None:
                    engines = ([nc.gpsimd, nc.scalar, nc.sync]
                               if b_stream else [nc.gpsimd, nc.scalar])
                for ci, c0 in enumerate(range(0, kpb, WB_CHUNK)):
                    cn = min(WB_CHUNK, kpb - c0)
                    eng = engines[ci % len(engines)]
                    eng.dma_start(
                        wbt[:, c0 : c0 + cn, :, :],
                        wb_d.ap()[s, :, c0 : c0 + cn, :, :],
                    )
                wb_tiles[s] = wbt

            # ---- Phase A: fp8 DoubleRow gate_up matmul (transposed) + swiglu -> h8 ----
            with (
                tc.tile_pool(name="pa", bufs=1) as pa,
                tc.tile_pool(name="wa_pool", bufs=wap_bufs) as wap,
                tc.tile_pool(name="psA", bufs=ps_bufs, space="PSUM") as psA,
                tc.tile_pool(name="swi", bufs=swi_bufs) as swi,
            ):
                xT = pa.tile([128, kpa, 2, tc_tokens], FP8)
                xch = min(4, kpa)
                for xc in range(0, kpa, xch):
                    eng = nc.sync if (xc // xch) % 2 == 0 else nc.gpsimd
                    eng.dma_start(xT[:, xc : xc + xch, :, :],
                                  x_d.ap()[:, xc : xc + xch, :, :])
                # a_reuse>1: timing-only probe — same MM count, 1/a_reuse the
                # LDWEIGHTS (each stationary streamed a_reuse times)
                hbn_a = (hbn // a_reuse) if a_reuse > 1 else hbn
                for hb in range(hbn_a if do_phases[0] else 0):
                    wsh = ([128, 2, kpa, 128] if a_jstride == 1
                           else [128, kpa // 4, 2, 4, 128] if a_jstride == 2
                           else [128, kpa, 2, 128])
                    wat_g = wap.tile(wsh, FP8, tag="wa")
                    wat_u = wap.tile(wsh, FP8, tag="wa")
                    if wa3ring and not (b_stream and hbn - 12 <= hb <= hbn - 5):
                        # 3-way split outside the wb-prefetch window
                        c3 = wsh[1] // 4
                        for wat, g in ((wat_g, 0), (wat_u, 1)):
                            nc.sync.dma_start(wat[:, : c3], wa_d.ap()[hb, :, g, : c3])
                            nc.gpsimd.dma_start(wat[:, c3 : 2 * c3], wa_d.ap()[hb, :, g, c3 : 2 * c3])
                            nc.scalar.dma_start(wat[:, 2 * c3 :], wa_d.ap()[hb, :, g, 2 * c3 :])
                    else:
                        nc.sync.dma_start(wat_g[:, : wsh[1] // 2], wa_d.ap()[hb, :, 0, : wsh[1] // 2])
                        nc.gpsimd.dma_start(wat_g[:, wsh[1] // 2 :], wa_d.ap()[hb, :, 0, wsh[1] // 2 :])
                        nc.gpsimd.dma_start(wat_u[:, : wsh[1] // 2], wa_d.ap()[hb, :, 1, : wsh[1] // 2])
                        nc.sync.dma_start(wat_u[:, wsh[1] // 2 :], wa_d.ap()[hb, :, 1, wsh[1] // 2 :])
                    if b_stream:
                        # prefetch the first two down slabs on the quiet
                        # ACT/DVE rings so phase B starts with both resident
                        if hb == hbn - 12:
                            issue_wb(0, engines=[nc.scalar])
                        elif hb == hbn - 6:
                            issue_wb(1, engines=[nc.scalar])
                    elif hb == hbn - 8:
                        # prefetch first down slab on the quiet ACT ring so it
                        # cannot stall the phase-A weight streams
                        issue_wb(0, engines=[nc.scalar])
                    ps_g = psA.tile([128, tc_tokens], F32, tag="psA")
                    ps_u = psA.tile([128, tc_tokens], F32, tag="psA")
                    pairs = ((wat_g, ps_g), (wat_u, ps_u))
                    if a_interleave:
                        order = [(wat, ps, kp) for kp in range(kpa)
                                 for wat, ps in pairs]
                    else:
                        order = [(wat, ps, kp) for wat, ps in pairs
                                 for kp in range(kpa)]
                    for wat, ps, kp in order:
                        lhsT = (wat[:, :, kp, :] if a_jstride == 1
                                else wat[:, kp // 4, :, kp % 4, :] if a_jstride == 2
                                else wat[:, kp, :, :])
                        for r in range(a_reuse):
                            nc.tensor.matmul(
                                ps[:],
                                lhsT=lhsT,
                                rhs=xT[:, kp, :, :],
                                start=(kp == 0 and r == 0),
                                stop=(kp == kpa - 1 and r == a_reuse - 1),
                                perf_mode=(mybir.MatmulPerfMode.DoubleRowSwInterleave
                                           if a_swi else DR),
                            )
                    if a_skip_swiglu:
                        junk = swi.tile([128, tc_tokens], F32, tag="junk")
                        nc.vector.tensor_tensor(
                            out=junk[:], in0=ps_g[:], in1=ps_u[:], op=ALU.mult)
                        continue
                    # SwiGLU on the [i(128), t(tc)] psum pair; psum carries
                    # gate_up * 2^9 (weight fold), descale rides the chain:
                    #   gc = min(ps_g*D^2, 7*D)        = D*min(g,7)
                    #   sg = sigmoid(gc * 1/D)         = sigmoid(min(g,7))
                    #   uc = clip(ps_u, +-7/D)         = (1/D)*clip(u,+-7)
                    #   gc = gc*sg;  h8 = gc*uc        = silu(min(g,7))*clip(u,+-7)
                    gc = swi.tile([128, tc_tokens], F32, tag="gc")
                    nc.vector.tensor_scalar(
                        out=gc[:], in0=ps_g[:],
                        scalar1=DESCALE * DESCALE, scalar2=LIMIT * DESCALE,
                        op0=ALU.mult, op1=ALU.min,
                    )
                    sg = swi.tile([128, tc_tokens], F32, tag="sg")
                    nc.scalar.activation(sg[:], gc[:], AF.Sigmoid, scale=WSHIFT)
                    uc = swi.tile([128, tc_tokens], F32, tag="uc")
                    nc.vector.tensor_scalar(
                        out=uc[:], in0=ps_u[:],
                        scalar1=LIMIT * WSHIFT, scalar2=-LIMIT * WSHIFT,
                        op0=ALU.min, op1=ALU.max,
                    )
                    nc.vector.tensor_mul(gc[:], gc[:], sg[:])
                    nc.vector.tensor_tensor(
                        out=h8[:, hb, :], in0=gc[:], in1=uc[:], op=ALU.mult,
                    )

            # ---- Phase B: fp8 DoubleRow down matmul ----
            with tc.tile_pool(name="psB", bufs=ps_bufs, space="PSUM") as psB:
                for s in range(slabs_b if do_phases[1] else 0):
                    if s not in wb_tiles:
                        issue_wb(s)
                    if s + 1 < slabs_b and (s + 1) not in wb_tiles:
                        issue_wb(s + 1)
                    wbt = wb_tiles.pop(s)
                    for tt in range(ntt):
                        ps = psB.tile([128, 512], F32, tag="psB")
                        for kp in range(kpb):
                            nc.tensor.matmul(
                                ps[:],
                                lhsT=h8[:, 2 * kp : 2 * kp + 2, ts(tt, 128)],
                                rhs=wbt[:, kp, :, :],
                                start=(kp == 0), stop=(kp == kpb - 1),
                                perf_mode=DR,
                            )
                        ot = oev.tile([128, 512], F32, tag="ot")
                        nc.scalar.activation(
                            ot[:], ps[:], AF.Copy, bias=0.0, scale=DESCALE,
                        )
                        nc.sync.dma_start(out_d.ap()[ts(tt, 128), ts(s, 512)], ot[:])

    if waitfix:
        from waitfix import split_multi_waits
        split_multi_waits(nc)
    return nc


# waitfix inlined so kernel.py stays self-contained
import sys as _sys
import types as _types

if "waitfix" not in _sys.modules:
    _wf = _types.ModuleType("waitfix")

    def _split_multi_waits(nc, limit: int = 1) -> int:
        n_split = 0
        f = nc.m.functions[0]
        for blk in f.blocks:
            insts = blk.instructions  # live list
            i = 0
            while i < len(insts):
                ins = insts[i]
                si = ins.sync_info
                if si is not None and len(si.on_wait) > limit:
                    waits = list(si.on_wait)
                    keep = waits[-limit:]
                    extra = waits[:-limit]
                    new_nops = []
                    for w in extra:
                        nop = mybir.InstNoOp(name=f"WSPLIT-{nc.next_id()}", ins=[], outs=[])
                        nop.engine = ins.engine
                        nop.sync_info = mybir.SyncInfo(on_wait=[w], on_update=[])
                        new_nops.append(nop)
                    ins.sync_info = mybir.SyncInfo(on_wait=keep, on_update=list(si.on_update))
                    for j, nop in enumerate(new_nops):
                        insts.insert(i + j, nop)
                    i += len(new_nops)
                    n_split += 1
                i += 1
        return n_split

    _wf.split_multi_waits = _split_multi_waits
    _sys.modules["waitfix"] = _wf


def _fold_w_fp8(w, s, block=BLOCK):
    """w [O, K] f32 (e4m3-representable values), s [O/128, K/128] pow-2 scales
    -> e4m3(w * s * 2^9) as float8_e4m3 [O, K].  Mantissa-exact shift."""
    ob, ib = s.shape
    w4 = w.reshape(ob, block, ib, block) * (s[:, None, :, None] * WSHIFT)
    w4 = w4.reshape(ob * block, ib * block)
    return np.clip(w4, -240.0, 240.0).astype(E4NP)


def _act_qdq_bf16(x):
    """Bit-exact replica of reference._act_qdq (jax cpu ops), cast to bf16
    (exact: e4m3 value x power-of-2 scale fits bf16's 8-bit mantissa)."""
    import jax
    import jax.numpy as jnp
    cpu = jax.devices("cpu")[0]
    with jax.default_device(cpu):
        xb = jnp.asarray(x, jnp.float32).reshape(-1, x.shape[-1] // BLOCK, BLOCK)
        amax = jnp.maximum(jnp.max(jnp.abs(xb), axis=-1), 1e-4)
        scale = jnp.exp2(jnp.ceil(jnp.log2(amax / FP8_MAX)))
        q = jnp.clip(xb / scale[..., None], -FP8_MAX, FP8_MAX)
        q = q.astype(jnp.float8_e4m3fn).astype(jnp.float32)
        out = np.asarray(q * scale[..., None]).reshape(x.shape)
    return out.astype(BF16NP)


def prep_weights(w_gate_up, s_gate_up, w_down, s_down, h=H, i_dim=I):
    """Host-side layouts (see build_nc docstring)."""
    hbn = i_dim // 128
    kpa = h // 256
    kpb = hbn // 2
    slabs_b = h // 512

    wA8 = _fold_w_fp8(w_gate_up, s_gate_up)             # [2I, H] fp8
    # [gu, hb, i, kp, j, p] -> [hb, p, gu, kp, j, i]
    wa = wA8.reshape(2, hbn, 128, kpa, 2, 128).transpose(1, 5, 0, 3, 4, 2)
    wa = np.ascontiguousarray(wa)

    wB8 = _fold_w_fp8(w_down, s_down)                   # [H, I] fp8
    # [s, o, kp, j, p] -> [s, p, kp, j, o]
    wb = wB8.reshape(slabs_b, 512, kpb, 2, 128).transpose(0, 4, 2, 3, 1)
    wb = np.ascontiguousarray(wb)
    return wa, wb


def prep_x(xq, h=H):
    """qdq'd x [T', h] bf16 -> e4m3 (exact for normal-range values) in the
    DR-paired [128, h/256, 2, T'] transposed layout."""
    tcn = xq.shape[0]
    x8 = xq.astype(np.float32).astype(E4NP)
    return np.ascontiguousarray(
        x8.reshape(tcn, h // 256, 2, 128).transpose(3, 1, 2, 0))


def prep_inputs(inputs):
    """Full input dict -> per-core in_maps."""
    x = np.asarray(inputs["x"], np.float32)
    xq = _act_qdq_bf16(x)
    wa, wb = prep_weights(
        np.asarray(inputs["w_gate_up"], np.float32),
        np.asarray(inputs["s_gate_up"], np.float32),
        np.asarray(inputs["w_down"], np.float32),
        np.asarray(inputs["s_down"], np.float32),
    )
    return [
        {"x": prep_x(xq[c * TC : (c + 1) * TC]), "wa": wa, "wb": wb}
        for c in range(N_CORES)
    ]


_CACHE = {}


def kernel(x, w_gate_up, s_gate_up, w_down, s_down):
    in_maps = prep_inputs(dict(x=x, w_gate_up=w_gate_up, s_gate_up=s_gate_up,
                               w_down=w_down, s_down=s_down))
    if "nc" not in _CACHE:
        _CACHE["nc"] = build_nc()
    nc = _CACHE["nc"]
    res = bass_utils.run_bass_kernel_spmd(nc, in_maps, core_ids=list(range(N_CORES)))
    return np.concatenate([res.results[c]["out"] for c in range(N_CORES)], axis=0)
